# revision 7
# baseline (speedup 1.0000x reference)
# Distributed Bass kernel for AttentionBlock: GroupNorm -> QKV 1x1 conv ->
# 4-head attention over L=4096 -> proj -> residual. N=2, C=256, H=W=64.
#
# Sharding: 8 cores = (n in 2) x (query chunk in 4). Each core computes the
# full output for 1024 query positions of one batch element. Softmax over keys
# is permutation-invariant, so each core receives x with the key/query axis
# rolled so that its own query chunk always sits at columns 0:1024 — all cores
# run the identical SPMD program. No collectives needed.
#
# Numerics: matmul path (xn, q, k, vT, probs, proj) in bf16 with fp32 PSUM
# accumulation; groupnorm stats, softmax denominators, residual in fp32.
# k-bias is dropped (cancels in softmax); v-bias is folded into the proj bias.
import sys

sys.path.insert(0, "/opt/trn_rl_repo")

import numpy as np

N, C, H, W = 2, 256, 64, 64
L = H * W          # 4096
NH, HD = 4, 64     # heads, head dim
G = 8              # groupnorm groups
GSIZE = C // G     # 32 channels per group (= 32 partitions per group)
QCHUNK = L // 4    # 1024 queries per core
NCORES = 8
EPS = 1e-5
KB = L // 128      # 32 key blocks of 128

_CACHE = {}


def _build_graph():
    import concourse.tile as tile
    from concourse import bacc, mybir
    from contextlib import ExitStack

    f32 = mybir.dt.float32
    bf16 = mybir.dt.bfloat16
    AF = mybir.ActivationFunctionType
    ALU = mybir.AluOpType
    AX = mybir.AxisListType

    nc = bacc.Bacc("TRN2")

    x_ext = nc.declare_dram_parameter("xbf", [128, 2, L], bf16, isOutput=False)
    xres_ext = nc.declare_dram_parameter("xres", [128, 2, QCHUNK], f32, isOutput=False)
    w_ext = nc.declare_dram_parameter("wt", [128, 2, 3 * C], bf16, isOutput=False)
    pw_ext = nc.declare_dram_parameter("pwt", [128, 2, C], bf16, isOutput=False)
    cst_ext = nc.declare_dram_parameter("cst", [128, 2, 4], f32, isOutput=False)
    sel_ext = nc.declare_dram_parameter("selg", [128, 4], f32, isOutput=False)
    selt_ext = nc.declare_dram_parameter("selgT", [4, 128], f32, isOutput=False)
    ones_ext = nc.declare_dram_parameter("ones64", [1, HD], f32, isOutput=False)
    out_ext = nc.declare_dram_parameter("out", [128, 2, QCHUNK], f32, isOutput=True)

    with tile.TileContext(nc) as tc, ExitStack() as ctx:
        big = ctx.enter_context(tc.tile_pool(name="big", bufs=1))
        small = ctx.enter_context(tc.tile_pool(name="small", bufs=1))
        expp = ctx.enter_context(tc.tile_pool(name="expp", bufs=3))
        rcpp = ctx.enter_context(tc.tile_pool(name="rcpp", bufs=2))
        psum = ctx.enter_context(tc.tile_pool(name="psum", bufs=2, space="PSUM"))
        psumo = ctx.enter_context(tc.tile_pool(name="psumo", bufs=2, space="PSUM"))

        # ---------------- input DMA ----------------
        x_sb = big.tile([128, 2, L], bf16)  # becomes xn in place
        nc.sync.dma_start(out=x_sb, in_=x_ext[:])
        xres = big.tile([128, 2, QCHUNK], f32)
        nc.sync.dma_start(out=xres, in_=xres_ext[:])
        w_sb = big.tile([128, 2, 3 * C], bf16)
        nc.sync.dma_start(out=w_sb, in_=w_ext[:])
        pw_sb = big.tile([128, 2, C], bf16)
        nc.sync.dma_start(out=pw_sb, in_=pw_ext[:])
        cst = small.tile([128, 2, 4], f32)  # [:, :, 0]=q_b, 1=norm_w, 2=norm_b, 3=p_eff
        nc.sync.dma_start(out=cst, in_=cst_ext[:])
        selg = small.tile([128, 4], f32)
        nc.sync.dma_start(out=selg, in_=sel_ext[:])
        selgT = small.tile([4, 128], f32)
        nc.sync.dma_start(out=selgT, in_=selt_ext[:])
        ones64 = small.tile([1, HD], f32)
        nc.sync.dma_start(out=ones64, in_=ones_ext[:])

        k_sb = big.tile([128, 2, L], bf16)  # [:, 0, :] doubles as Square scratch

        # ---------------- groupnorm stats ----------------
        # per-(partition, co) sums over L; groups are 32-partition blocks.
        st = small.tile([128, 4], f32)  # [:, 0:2]=sum, [:, 2:4]=sumsq
        nc.vector.tensor_reduce(out=st[:, 0:2], in_=x_sb, axis=AX.X, op=ALU.add)
        for co in range(2):
            nc.scalar.activation(
                out=k_sb[:, 0, :],
                in_=x_sb[:, co, :],
                func=AF.Square,
                accum_out=st[:, 2 + co : 3 + co],
            )
        # consolidate st (ACT+DVE writers) through one DVE copy so the matmul
        # below needs at most 2 sync waits (walrus per-instruction limit)
        st2 = small.tile([128, 4], f32)
        nc.vector.tensor_copy(out=st2, in_=st)
        # group totals: [4 groups-in-co, (sum0, sum1, sq0, sq1)]
        gstp = psumo.tile([128, 512], f32, tag="o")
        nc.tensor.matmul(gstp[0:4, 0:4], lhsT=selg, rhs=st2, start=True, stop=True)
        inv_cnt = 1.0 / (GSIZE * L)
        mr = small.tile([4, 4], f32)  # cols 0:2 mean, 2:4 rstd
        nc.vector.tensor_scalar_mul(mr[:, 0:2], gstp[0:4, 0:4][:, 0:2], inv_cnt)
        nc.vector.tensor_scalar_mul(mr[:, 2:4], gstp[0:4, 0:4][:, 2:4], inv_cnt)
        musq = small.tile([4, 2], f32)
        nc.vector.tensor_mul(musq, mr[:, 0:2], mr[:, 0:2])
        nc.vector.tensor_sub(mr[:, 2:4], mr[:, 2:4], musq)  # var
        eps_sb = small.tile([4, 1], f32)
        nc.vector.memset(eps_sb, EPS)
        nc.scalar.activation(out=mr[:, 2:4], in_=mr[:, 2:4], func=AF.Sqrt, bias=eps_sb)
        nc.vector.reciprocal(mr[:, 2:4], mr[:, 2:4])  # rstd
        # broadcast group stats to channels: [128, (mu0, mu1, rs0, rs1)]
        chp = psumo.tile([128, 512], f32, tag="o")
        nc.tensor.matmul(chp[:, 0:4], lhsT=selgT, rhs=mr, start=True, stop=True)
        # scale = rstd * norm_w ; shift = norm_b - mu * scale
        sc = small.tile([128, 4], f32)  # cols 0:2 scale, 2:4 shift
        nc.vector.tensor_mul(sc[:, 0:2], chp[:, 2:4], cst[:, :, 1])
        tmp_ms = small.tile([128, 2], f32)
        nc.vector.tensor_mul(tmp_ms, chp[:, 0:2], sc[:, 0:2])
        nc.vector.tensor_sub(sc[:, 2:4], cst[:, :, 2], tmp_ms)

        # xn in place: co=0 on scalar engine, co=1 on vector engine
        nc.scalar.activation(
            out=x_sb[:, 0, :],
            in_=x_sb[:, 0, :],
            func=AF.Identity,
            bias=sc[:, 2:3],
            scale=sc[:, 0:1],
        )
        nc.vector.tensor_scalar(
            out=x_sb[:, 1, :],
            in0=x_sb[:, 1, :],
            scalar1=sc[:, 1:2],
            scalar2=sc[:, 3:4],
            op0=ALU.mult,
            op1=ALU.add,
        )

        # ---------------- QKV projections ----------------
        q_sb = big.tile([128, 2, QCHUNK], bf16)
        vT = big.tile([128, KB, NH, HD + 1], bf16)  # [key-part, kb, head, chan+ones]
        nc.vector.memset(vT[:, :, :, HD : HD + 1], 1.0)

        # q for own chunk (+bias)
        for co in range(2):
            for t in range(QCHUNK // 512):
                ps = psum.tile([128, 512], f32, tag="s")
                for ki in range(2):
                    nc.tensor.matmul(
                        ps,
                        lhsT=w_sb[:, ki, co * 128 : (co + 1) * 128],
                        rhs=x_sb[:, ki, t * 512 : (t + 1) * 512],
                        start=(ki == 0),
                        stop=(ki == 1),
                    )
                nc.vector.tensor_scalar_add(
                    out=q_sb[:, co, t * 512 : (t + 1) * 512],
                    in0=ps,
                    scalar1=cst[:, co, 0:1],
                )
        # k full (no bias: cancels in softmax)
        for co in range(2):
            for t in range(L // 512):
                ps = psum.tile([128, 512], f32, tag="s")
                for ki in range(2):
                    nc.tensor.matmul(
                        ps,
                        lhsT=w_sb[:, ki, C + co * 128 : C + (co + 1) * 128],
                        rhs=x_sb[:, ki, t * 512 : (t + 1) * 512],
                        start=(ki == 0),
                        stop=(ki == 1),
                    )
                nc.vector.tensor_copy(out=k_sb[:, co, t * 512 : (t + 1) * 512], in_=ps)
        # vT = v transposed, computed directly: [key, chan] (v bias folded into p_eff)
        for kb in range(KB):
            ps = psum.tile([128, 512], f32, tag="s")
            for ki in range(2):
                nc.tensor.matmul(
                    ps[:, 0:256],
                    lhsT=x_sb[:, ki, kb * 128 : (kb + 1) * 128],
                    rhs=w_sb[:, ki, 2 * C : 3 * C],
                    start=(ki == 0),
                    stop=(ki == 1),
                )
            nc.vector.tensor_copy(
                out=vT[:, kb, :, 0:HD],
                in_=ps[:, 0:256].rearrange("p (h c) -> p h c", h=NH),
            )

        # ---------------- attention (flash-style, per head) ----------------
        out_attn = big.tile([128, 2, QCHUNK], bf16)
        for h in range(NH):
            co_h, r0 = h // 2, (h % 2) * 64
            o = psumo.tile([HD + 1, QCHUNK], f32, tag="o")  # row 64 = denom
            for kb in range(KB):
                s = psum.tile([128, QCHUNK], f32, tag="s")
                for t in range(QCHUNK // 512):
                    nc.tensor.matmul(
                        s[:, t * 512 : (t + 1) * 512],
                        lhsT=k_sb[r0 : r0 + 64, co_h, kb * 128 : (kb + 1) * 128],
                        rhs=q_sb[r0 : r0 + 64, co_h, t * 512 : (t + 1) * 512],
                        start=True,
                        stop=True,
                    )
                e = expp.tile([128, QCHUNK], bf16)
                nc.scalar.activation(out=e, in_=s, func=AF.Exp, scale=0.125)
                for t in range(QCHUNK // 512):
                    nc.tensor.matmul(
                        o[:, t * 512 : (t + 1) * 512],
                        lhsT=vT[:, kb, h, :],
                        rhs=e[:, t * 512 : (t + 1) * 512],
                        start=(kb == 0),
                        stop=(kb == KB - 1),
                    )
            # normalize: out = o[0:64] * (1/denom) broadcast over channels
            rcp = rcpp.tile([1, QCHUNK], f32, tag="rcp")
            nc.vector.reciprocal(rcp, o[HD : HD + 1, :])
            rbp = psum.tile([128, QCHUNK], f32, tag="s")
            for t in range(QCHUNK // 512):
                nc.tensor.matmul(
                    rbp[0:64, t * 512 : (t + 1) * 512],
                    lhsT=ones64,
                    rhs=rcp[:, t * 512 : (t + 1) * 512],
                    start=True,
                    stop=True,
                )
            rb = rcpp.tile([64, QCHUNK], f32, tag="rb")
            nc.vector.tensor_copy(out=rb, in_=rbp[0:64, :])
            nc.vector.tensor_mul(out_attn[r0 : r0 + 64, co_h, :], o[0:HD, :], rb)

        # ---------------- projection + bias + residual ----------------
        final = big.tile([128, 2, QCHUNK], f32)
        for co in range(2):
            for t in range(QCHUNK // 512):
                ps = psum.tile([128, 512], f32, tag="s")
                for ki in range(2):
                    nc.tensor.matmul(
                        ps,
                        lhsT=pw_sb[:, ki, co * 128 : (co + 1) * 128],
                        rhs=out_attn[:, ki, t * 512 : (t + 1) * 512],
                        start=(ki == 0),
                        stop=(ki == 1),
                    )
                nc.vector.scalar_tensor_tensor(
                    out=final[:, co, t * 512 : (t + 1) * 512],
                    in0=ps,
                    scalar=cst[:, co, 3:4],
                    in1=xres[:, co, t * 512 : (t + 1) * 512],
                    op0=ALU.add,
                    op1=ALU.add,
                )
        nc.sync.dma_start(out=out_ext[:], in_=final)

    nc.finalize()
    return nc


def _prep_inputs(x, q_w, k_w, v_w, p_w, q_b, v_b, p_b, norm_w, norm_b):
    """Build per-core input maps (all host-side numpy)."""
    import ml_dtypes

    bf = ml_dtypes.bfloat16
    qkvw = np.concatenate([q_w.T, k_w.T, v_w.T], axis=1)  # [C, 3C]
    wt = np.ascontiguousarray(qkvw.reshape(2, 128, 3 * C).transpose(1, 0, 2)).astype(bf)
    pwt = np.ascontiguousarray(p_w.T.reshape(2, 128, C).transpose(1, 0, 2)).astype(bf)
    p_eff = p_b + p_w @ v_b  # v bias folded through the (sum-to-1) softmax
    cstv = np.stack([q_b, norm_w, norm_b, p_eff], axis=1)  # [C, 4]
    cst = np.ascontiguousarray(cstv.reshape(2, 128, 4).transpose(1, 0, 2)).astype(
        np.float32
    )
    selg = np.zeros((128, 4), np.float32)
    for ci in range(128):
        selg[ci, ci // GSIZE] = 1.0
    selgT = np.ascontiguousarray(selg.T)
    ones64 = np.ones((1, HD), np.float32)

    in_maps = []
    for core in range(NCORES):
        n, qc = core // 4, core % 4
        x_n = x[n].reshape(C, L)
        x_roll = np.roll(x_n, -qc * QCHUNK, axis=1)
        x_host = np.ascontiguousarray(x_roll.reshape(2, 128, L).transpose(1, 0, 2))
        in_maps.append(
            {
                "xbf": x_host.astype(bf),
                "xres": x_host[:, :, 0:QCHUNK].astype(np.float32),
                "wt": wt,
                "pwt": pwt,
                "cst": cst,
                "selg": selg,
                "selgT": selgT,
                "ones64": ones64,
            }
        )
    return in_maps


def kernel(x, mask, norm_w, norm_b, q_w, q_b, k_w, k_b, v_w, v_b, p_w, p_b):
    x = np.asarray(x, np.float32)
    mask = np.asarray(mask)
    args = [np.asarray(a, np.float32) for a in (norm_w, norm_b, q_w, q_b, k_w, k_b, v_w, v_b, p_w, p_b)]
    norm_w, norm_b, q_w, q_b, k_w, k_b, v_w, v_b, p_w, p_b = args

    from concourse.bass_utils import run_bass_kernel_spmd

    if "nc" not in _CACHE:
        _CACHE["nc"] = _build_graph()
    nc = _CACHE["nc"]

    in_maps = _prep_inputs(x, q_w, k_w, v_w, p_w, q_b, v_b, p_b, norm_w, norm_b)
    res = run_bass_kernel_spmd(nc, in_maps, core_ids=list(range(NCORES)))
    results = res.results

    y = np.empty((N, C, L), np.float32)
    for core in range(NCORES):
        n, qc = core // 4, core % 4
        arr = np.asarray(results[core]["out"], np.float32)  # [128, 2, QCHUNK]
        y[n][:, qc * QCHUNK : (qc + 1) * QCHUNK] = arr.transpose(1, 0, 2).reshape(
            C, QCHUNK
        )
    return (y.reshape(N, C, H, W), mask)


# revision 10
# speedup vs baseline: 1.3189x; 1.3189x over previous
# Distributed Bass kernel for AttentionBlock: GroupNorm -> QKV 1x1 conv ->
# 4-head attention over L=4096 -> proj -> residual. N=2, C=256, H=W=64.
#
# Sharding: 8 cores = (n in 2) x (query chunk in 4). Each core computes the
# full output for 1024 query positions of one batch element. Softmax over keys
# is permutation-invariant, so each core receives x with the key/query axis
# rolled so that its own query chunk always sits at columns 0:1024 — all cores
# run the identical SPMD program. No collectives needed.
#
# Numerics: matmul path (xn, q, k, vT, probs, proj) in bf16 with fp32 PSUM
# accumulation; groupnorm stats, softmax denominators, residual in fp32.
# k-bias is dropped (cancels in softmax); v-bias is folded into the proj bias.
import sys

sys.path.insert(0, "/opt/trn_rl_repo")

import numpy as np

N, C, H, W = 2, 256, 64, 64
L = H * W          # 4096
NH, HD = 4, 64     # heads, head dim
G = 8              # groupnorm groups
GSIZE = C // G     # 32 channels per group (= 32 partitions per group)
QCHUNK = L // 4    # 1024 queries per core
NCORES = 8
EPS = 1e-5
KB = L // 128      # 32 key blocks of 128

_CACHE = {}


def _build_graph():
    import concourse.tile as tile
    from concourse import bacc, mybir
    from contextlib import ExitStack

    f32 = mybir.dt.float32
    bf16 = mybir.dt.bfloat16
    AF = mybir.ActivationFunctionType
    ALU = mybir.AluOpType
    AX = mybir.AxisListType

    nc = bacc.Bacc("TRN2")

    x_ext = nc.declare_dram_parameter("xbf", [128, 2, L], bf16, isOutput=False)
    xres_ext = nc.declare_dram_parameter("xres", [128, 2, QCHUNK], f32, isOutput=False)
    w_ext = nc.declare_dram_parameter("wt", [128, 2, 3 * C], bf16, isOutput=False)
    pw_ext = nc.declare_dram_parameter("pwt", [128, 2, C], bf16, isOutput=False)
    cst_ext = nc.declare_dram_parameter("cst", [128, 2, 4], f32, isOutput=False)
    sel_ext = nc.declare_dram_parameter("selg", [128, 4], f32, isOutput=False)
    selt_ext = nc.declare_dram_parameter("selgT", [4, 128], f32, isOutput=False)
    ones_ext = nc.declare_dram_parameter("ones64", [1, HD], f32, isOutput=False)
    out_ext = nc.declare_dram_parameter("out", [128, 2, QCHUNK], f32, isOutput=True)

    with tile.TileContext(nc) as tc, ExitStack() as ctx:
        big = ctx.enter_context(tc.tile_pool(name="big", bufs=1))
        small = ctx.enter_context(tc.tile_pool(name="small", bufs=1))
        expp = ctx.enter_context(tc.tile_pool(name="expp", bufs=3))
        rcpp = ctx.enter_context(tc.tile_pool(name="rcpp", bufs=2))
        psum = ctx.enter_context(tc.tile_pool(name="psum", bufs=2, space="PSUM"))
        psumo = ctx.enter_context(tc.tile_pool(name="psumo", bufs=4, space="PSUM"))

        # ---------------- input DMA ----------------
        x_sb = big.tile([128, 2, L], bf16)  # becomes xn in place
        nc.sync.dma_start(out=x_sb, in_=x_ext[:])
        xres = big.tile([128, 2, QCHUNK], f32)
        nc.sync.dma_start(out=xres, in_=xres_ext[:])
        w_sb = big.tile([128, 2, 3 * C], bf16)
        nc.sync.dma_start(out=w_sb, in_=w_ext[:])
        pw_sb = big.tile([128, 2, C], bf16)
        nc.sync.dma_start(out=pw_sb, in_=pw_ext[:])
        cst = small.tile([128, 2, 4], f32)  # [:, :, 0]=q_b, 1=norm_w, 2=norm_b, 3=p_eff
        nc.sync.dma_start(out=cst, in_=cst_ext[:])
        selg = small.tile([128, 4], f32)
        nc.sync.dma_start(out=selg, in_=sel_ext[:])
        selgT = small.tile([4, 128], f32)
        nc.sync.dma_start(out=selgT, in_=selt_ext[:])
        ones64 = small.tile([1, HD], f32)
        nc.sync.dma_start(out=ones64, in_=ones_ext[:])

        k_sb = big.tile([128, 2, L], bf16)  # [:, 0, :] doubles as Square scratch

        # ---------------- groupnorm stats ----------------
        # per-(partition, co) sums over L; groups are 32-partition blocks.
        st = small.tile([128, 4], f32)  # [:, 0:2]=sum, [:, 2:4]=sumsq
        nc.vector.tensor_reduce(out=st[:, 0:2], in_=x_sb, axis=AX.X, op=ALU.add)
        for co in range(2):
            nc.scalar.activation(
                out=k_sb[:, 0, :],
                in_=x_sb[:, co, :],
                func=AF.Square,
                accum_out=st[:, 2 + co : 3 + co],
            )
        # consolidate st (ACT+DVE writers) through one DVE copy so the matmul
        # below needs at most 2 sync waits (walrus per-instruction limit)
        st2 = small.tile([128, 4], f32)
        nc.vector.tensor_copy(out=st2, in_=st)
        # group totals: [4 groups-in-co, (sum0, sum1, sq0, sq1)]
        gstp = psumo.tile([128, 512], f32, tag="o")
        nc.tensor.matmul(gstp[0:4, 0:4], lhsT=selg, rhs=st2, start=True, stop=True)
        inv_cnt = 1.0 / (GSIZE * L)
        mr = small.tile([4, 4], f32)  # cols 0:2 mean, 2:4 rstd
        nc.vector.tensor_scalar_mul(mr[:, 0:2], gstp[0:4, 0:4][:, 0:2], inv_cnt)
        nc.vector.tensor_scalar_mul(mr[:, 2:4], gstp[0:4, 0:4][:, 2:4], inv_cnt)
        musq = small.tile([4, 2], f32)
        nc.vector.tensor_mul(musq, mr[:, 0:2], mr[:, 0:2])
        nc.vector.tensor_sub(mr[:, 2:4], mr[:, 2:4], musq)  # var
        eps_sb = small.tile([4, 1], f32)
        nc.vector.memset(eps_sb, EPS)
        nc.scalar.activation(out=mr[:, 2:4], in_=mr[:, 2:4], func=AF.Sqrt, bias=eps_sb)
        nc.vector.reciprocal(mr[:, 2:4], mr[:, 2:4])  # rstd
        # broadcast group stats to channels: [128, (mu0, mu1, rs0, rs1)]
        chp = psumo.tile([128, 512], f32, tag="o")
        nc.tensor.matmul(chp[:, 0:4], lhsT=selgT, rhs=mr, start=True, stop=True)
        # scale = rstd * norm_w ; shift = norm_b - mu * scale
        sc = small.tile([128, 4], f32)  # cols 0:2 scale, 2:4 shift
        nc.vector.tensor_mul(sc[:, 0:2], chp[:, 2:4], cst[:, :, 1])
        tmp_ms = small.tile([128, 2], f32)
        nc.vector.tensor_mul(tmp_ms, chp[:, 0:2], sc[:, 0:2])
        nc.vector.tensor_sub(sc[:, 2:4], cst[:, :, 2], tmp_ms)

        # xn in place: co=0 on scalar engine, co=1 on vector engine
        nc.scalar.activation(
            out=x_sb[:, 0, :],
            in_=x_sb[:, 0, :],
            func=AF.Identity,
            bias=sc[:, 2:3],
            scale=sc[:, 0:1],
        )
        nc.vector.tensor_scalar(
            out=x_sb[:, 1, :],
            in0=x_sb[:, 1, :],
            scalar1=sc[:, 1:2],
            scalar2=sc[:, 3:4],
            op0=ALU.mult,
            op1=ALU.add,
        )

        # ---------------- QKV projections ----------------
        q_sb = big.tile([128, 2, QCHUNK], bf16)
        vT = big.tile([128, KB, NH, HD + 1], bf16)  # [key-part, kb, head, chan+ones]
        nc.vector.memset(vT[:, :, :, HD : HD + 1], 1.0)

        # q for own chunk (+bias)
        for co in range(2):
            for t in range(QCHUNK // 512):
                ps = psum.tile([128, 512], f32, tag="s")
                for ki in range(2):
                    nc.tensor.matmul(
                        ps,
                        lhsT=w_sb[:, ki, co * 128 : (co + 1) * 128],
                        rhs=x_sb[:, ki, t * 512 : (t + 1) * 512],
                        start=(ki == 0),
                        stop=(ki == 1),
                    )
                nc.vector.tensor_scalar_add(
                    out=q_sb[:, co, t * 512 : (t + 1) * 512],
                    in0=ps,
                    scalar1=cst[:, co, 0:1],
                )
        # k full (no bias: cancels in softmax); copybacks alternate DVE/ACT
        for co in range(2):
            for t in range(L // 512):
                ps = psum.tile([128, 512], f32, tag="s")
                for ki in range(2):
                    nc.tensor.matmul(
                        ps,
                        lhsT=w_sb[:, ki, C + co * 128 : C + (co + 1) * 128],
                        rhs=x_sb[:, ki, t * 512 : (t + 1) * 512],
                        start=(ki == 0),
                        stop=(ki == 1),
                    )
                dst = k_sb[:, co, t * 512 : (t + 1) * 512]
                if t % 2 == 0:
                    nc.vector.tensor_copy(out=dst, in_=ps)
                else:
                    nc.scalar.copy(out=dst, in_=ps)
        # vT = v transposed, computed directly: [key, chan] (v bias folded into p_eff)
        for kb in range(KB):
            ps = psum.tile([128, 512], f32, tag="s")
            for ki in range(2):
                nc.tensor.matmul(
                    ps[:, 0:256],
                    lhsT=x_sb[:, ki, kb * 128 : (kb + 1) * 128],
                    rhs=w_sb[:, ki, 2 * C : 3 * C],
                    start=(ki == 0),
                    stop=(ki == 1),
                )
            src = ps[:, 0:256].rearrange("p (h c) -> p h c", h=NH)
            dst = vT[:, kb, :, 0:HD]
            if kb % 2 == 0:
                nc.vector.tensor_copy(out=dst, in_=src)
            else:
                nc.scalar.copy(out=dst, in_=src)

        # ---------------- attention (flash-style, head pairs) ----------------
        # Heads (2co, 2co+1) live on partitions 0:64 / 64:128 of k_sb[:, co] —
        # their score matmuls target distinct PE row groups and run
        # concurrently. One Exp covers both heads' scores (FD=1024), keeping
        # the ScalarE (the bottleneck) on big tiles.
        out_attn = big.tile([128, 2, QCHUNK], bf16)
        for co in range(2):
            for qt in range(QCHUNK // 512):
                qs = slice(qt * 512, (qt + 1) * 512)
                o0 = psumo.tile([HD + 1, 512], f32, tag="o")  # head 2co; row 64=denom
                o1 = psumo.tile([HD + 1, 512], f32, tag="o")  # head 2co+1
                for kb in range(KB):
                    ks = slice(kb * 128, (kb + 1) * 128)
                    s = psum.tile([128, 2, 512], f32, tag="s")
                    nc.tensor.matmul(
                        s[:, 0, :],
                        lhsT=k_sb[0:64, co, ks],
                        rhs=q_sb[0:64, co, qs],
                        start=True,
                        stop=True,
                    )
                    nc.tensor.matmul(
                        s[:, 1, :],
                        lhsT=k_sb[64:128, co, ks],
                        rhs=q_sb[64:128, co, qs],
                        start=True,
                        stop=True,
                    )
                    e = expp.tile([128, 2, 512], bf16)
                    nc.scalar.activation(out=e, in_=s, func=AF.Exp, scale=0.125)
                    nc.tensor.matmul(
                        o0,
                        lhsT=vT[:, kb, 2 * co, :],
                        rhs=e[:, 0, :],
                        start=(kb == 0),
                        stop=(kb == KB - 1),
                    )
                    nc.tensor.matmul(
                        o1,
                        lhsT=vT[:, kb, 2 * co + 1, :],
                        rhs=e[:, 1, :],
                        start=(kb == 0),
                        stop=(kb == KB - 1),
                    )
                # normalize: out = o[0:64] * (1/denom) broadcast over channels
                for h, o in ((2 * co, o0), (2 * co + 1, o1)):
                    r0 = (h % 2) * 64
                    rcp = rcpp.tile([1, 512], f32, tag="rcp")
                    nc.vector.reciprocal(rcp, o[HD : HD + 1, :])
                    rbp = psum.tile([128, 2, 512], f32, tag="s")
                    nc.tensor.matmul(
                        rbp[0:64, 0, :], lhsT=ones64, rhs=rcp, start=True, stop=True
                    )
                    rb = rcpp.tile([64, 512], f32, tag="rb")
                    nc.vector.tensor_copy(out=rb, in_=rbp[0:64, 0, :])
                    nc.vector.tensor_mul(
                        out_attn[r0 : r0 + 64, co, qs], o[0:HD, :], rb
                    )

        # ---------------- projection + bias + residual ----------------
        final = big.tile([128, 2, QCHUNK], f32)
        for co in range(2):
            for t in range(QCHUNK // 512):
                ps = psum.tile([128, 512], f32, tag="s")
                for ki in range(2):
                    nc.tensor.matmul(
                        ps,
                        lhsT=pw_sb[:, ki, co * 128 : (co + 1) * 128],
                        rhs=out_attn[:, ki, t * 512 : (t + 1) * 512],
                        start=(ki == 0),
                        stop=(ki == 1),
                    )
                nc.vector.scalar_tensor_tensor(
                    out=final[:, co, t * 512 : (t + 1) * 512],
                    in0=ps,
                    scalar=cst[:, co, 3:4],
                    in1=xres[:, co, t * 512 : (t + 1) * 512],
                    op0=ALU.add,
                    op1=ALU.add,
                )
        nc.sync.dma_start(out=out_ext[:], in_=final)

    nc.finalize()
    return nc


def _prep_inputs(x, q_w, k_w, v_w, p_w, q_b, v_b, p_b, norm_w, norm_b):
    """Build per-core input maps (all host-side numpy)."""
    import ml_dtypes

    bf = ml_dtypes.bfloat16
    qkvw = np.concatenate([q_w.T, k_w.T, v_w.T], axis=1)  # [C, 3C]
    wt = np.ascontiguousarray(qkvw.reshape(2, 128, 3 * C).transpose(1, 0, 2)).astype(bf)
    pwt = np.ascontiguousarray(p_w.T.reshape(2, 128, C).transpose(1, 0, 2)).astype(bf)
    p_eff = p_b + p_w @ v_b  # v bias folded through the (sum-to-1) softmax
    cstv = np.stack([q_b, norm_w, norm_b, p_eff], axis=1)  # [C, 4]
    cst = np.ascontiguousarray(cstv.reshape(2, 128, 4).transpose(1, 0, 2)).astype(
        np.float32
    )
    selg = np.zeros((128, 4), np.float32)
    for ci in range(128):
        selg[ci, ci // GSIZE] = 1.0
    selgT = np.ascontiguousarray(selg.T)
    ones64 = np.ones((1, HD), np.float32)

    in_maps = []
    for core in range(NCORES):
        n, qc = core // 4, core % 4
        x_n = x[n].reshape(C, L)
        x_roll = np.roll(x_n, -qc * QCHUNK, axis=1)
        x_host = np.ascontiguousarray(x_roll.reshape(2, 128, L).transpose(1, 0, 2))
        in_maps.append(
            {
                "xbf": x_host.astype(bf),
                "xres": x_host[:, :, 0:QCHUNK].astype(np.float32),
                "wt": wt,
                "pwt": pwt,
                "cst": cst,
                "selg": selg,
                "selgT": selgT,
                "ones64": ones64,
            }
        )
    return in_maps


def kernel(x, mask, norm_w, norm_b, q_w, q_b, k_w, k_b, v_w, v_b, p_w, p_b):
    x = np.asarray(x, np.float32)
    mask = np.asarray(mask)
    args = [np.asarray(a, np.float32) for a in (norm_w, norm_b, q_w, q_b, k_w, k_b, v_w, v_b, p_w, p_b)]
    norm_w, norm_b, q_w, q_b, k_w, k_b, v_w, v_b, p_w, p_b = args

    from concourse.bass_utils import run_bass_kernel_spmd

    if "nc" not in _CACHE:
        _CACHE["nc"] = _build_graph()
    nc = _CACHE["nc"]

    in_maps = _prep_inputs(x, q_w, k_w, v_w, p_w, q_b, v_b, p_b, norm_w, norm_b)
    res = run_bass_kernel_spmd(nc, in_maps, core_ids=list(range(NCORES)))
    results = res.results

    y = np.empty((N, C, L), np.float32)
    for core in range(NCORES):
        n, qc = core // 4, core % 4
        arr = np.asarray(results[core]["out"], np.float32)  # [128, 2, QCHUNK]
        y[n][:, qc * QCHUNK : (qc + 1) * QCHUNK] = arr.transpose(1, 0, 2).reshape(
            C, QCHUNK
        )
    return (y.reshape(N, C, H, W), mask)
